# revision 32
# baseline (speedup 1.0000x reference)
"""Trainium2 Bass kernel for a cosine-sim causal attention block (8 NeuronCores).

Reference computation (single device):
  y   = LayerNorm(x) * g + b
  qkv = y @ c_attn_w + c_attn_b ; split q,k,v into 16 heads of 64
  qn, kn = l2norm(q), l2norm(k)
  attn = softmax(causal(8 * qn @ kn^T))
  o    = attn @ v  (merge heads)
  o    = o @ c_proj_w + c_proj_b
  out  = o @ to_out_w.T

Sharding: 2 batch groups x 4 cores; core owns one batch and 4 heads
(tensor-parallel c_attn columns / c_proj rows).  Chunked ReduceScatter of the
c_proj partials inside each group token-shards the output; each core then
runs to_out on its 512-token shard.  Host folds g/b into c_attn, the v-bias
and c_proj bias through to a single to_out bias, and reassembles shards.

Compute: bf16 matmul operands, fp32 accumulation/softmax statistics.
The whole per-core computation is pipelined over four 512-token chunks
(LN -> qkv -> v -> l2norms -> attention -> c_proj -> reduce-scatter), with
per-chunk tiles so the Tile scheduler overlaps phases across chunks.
Attention runs in the transposed [k, q] layout so softmax + attn@v need no
on-chip transposes; softmax denominators come free from a ones-column
appended to v; q/k norms fold into broadcast multiplies; transposes go
through the DMA xbar in bf16.
"""

import sys

for _p in ("/opt/trn_rl_repo", "/root/.axon_site/_ro/trn_rl_repo"):
    if _p not in sys.path:
        sys.path.insert(0, _p)

import numpy as np
import ml_dtypes

from concourse import bass, bacc, tile, mybir
from concourse.bass_utils import run_bass_kernel_spmd

B, N, DIM, H, DH = 2, 2048, 1024, 16, 64
SCALE = 8.0
LN_EPS = 1e-5

N_CORES = 8
GROUPS = [[0, 1, 2, 3], [4, 5, 6, 7]]
HPC = 4            # heads per core
HD = HPC * DH      # 256 head dims per core
NSH = N // 4       # 512 tokens per core after reduce-scatter

F32 = mybir.dt.float32
BF = mybir.dt.bfloat16
AF = mybir.ActivationFunctionType
OP = mybir.AluOpType
AX = mybir.AxisListType

NT = N // 128      # 16 token tiles
KD = DIM // 128    # 8 dim tiles
VW = DH + 1        # per-head v block width incl ones column

DEBUG_DUMPS = False


def _build_nc():
    nc = bacc.Bacc(
        "TRN2", target_bir_lowering=False, debug=False, num_devices=N_CORES
    )

    xb = nc.dram_tensor("xb", [N, DIM], F32, kind="ExternalInput")
    wqk = nc.dram_tensor("wqk", [DIM, 512], BF, kind="ExternalInput")
    bqk = nc.dram_tensor("bqk", [512], F32, kind="ExternalInput")
    wv = nc.dram_tensor("wv", [DIM, HD], BF, kind="ExternalInput")
    wp = nc.dram_tensor("wp", [HD, DIM], BF, kind="ExternalInput")
    wto = nc.dram_tensor("wto", [DIM, DIM], BF, kind="ExternalInput")
    bout = nc.dram_tensor("bout", [DIM], F32, kind="ExternalInput")
    maskd = nc.dram_tensor("maskd", [128, 128], F32, kind="ExternalInput")
    hsel = nc.dram_tensor("hsel", [128, 2], BF, kind="ExternalInput")
    out_ext = nc.dram_tensor("out", [DIM, NSH], F32, kind="ExternalOutput")

    proj_c = [nc.dram_tensor(f"proj_b{c}", [NSH, DIM], BF) for c in range(4)]
    rs_c = [nc.dram_tensor(f"rs_b{c}", [128, DIM], BF) for c in range(4)]
    if DEBUG_DUMPS:
        d_qkT = nc.dram_tensor("d_qkT", [128, 4 * N], BF, kind="ExternalOutput")
        d_v = nc.dram_tensor("d_v", [128, NT * HPC * VW], BF, kind="ExternalOutput")
        d_oT = nc.dram_tensor("d_oT", [128, 2 * N], BF, kind="ExternalOutput")

    with tile.TileContext(nc) as tc:
        with (
            tc.tile_pool(name="wpool", bufs=1) as wpool,
            tc.tile_pool(name="small", bufs=1) as small,
            tc.tile_pool(name="ps", bufs=2, space="PSUM") as pspool,
            tc.tile_pool(name="pss", bufs=2, space="PSUM") as simpool,
            tc.tile_pool(name="pso", bufs=2, space="PSUM") as opspool,
        ):
            # ---- persistent weights ----
            w_qk = wpool.tile([128, KD * 512], BF, tag="wqk")
            nc.sync.dma_start(
                w_qk[:].rearrange("p (kt m) -> p kt m", kt=KD),
                wqk.ap().rearrange("(kt p) m -> p kt m", p=128),
            )
            w_v = wpool.tile([128, KD * HD], BF, tag="wv")
            nc.sync.dma_start(
                w_v[:].rearrange("p (kt m) -> p kt m", kt=KD),
                wv.ap().rearrange("(kt p) m -> p kt m", p=128),
            )
            w_p = wpool.tile([128, 2 * DIM], BF, tag="wp")
            nc.sync.dma_start(
                w_p[:].rearrange("p (kt m) -> p kt m", kt=2),
                wp.ap().rearrange("(kt p) m -> p kt m", p=128),
            )
            b_qk = small.tile([128, 4], F32, tag="bqk")
            nc.sync.dma_start(b_qk[:], bqk.ap().rearrange("(mt p) -> p mt", p=128))
            b_out = small.tile([128, 8], F32, tag="bout")
            nc.sync.dma_start(b_out[:], bout.ap().rearrange("(mt p) -> p mt", p=128))
            mask_sb = small.tile([128, 128], F32, tag="mask")
            nc.sync.dma_start(mask_sb[:], maskd.ap())
            hsel_sb = small.tile([128, 2], BF, tag="hsel")
            nc.sync.dma_start(hsel_sb[:], hsel.ap())

            eps_ln = small.tile([128, 1], F32, tag="epsln")
            nc.gpsimd.memset(eps_ln[:], LN_EPS)
            eps_n = small.tile([128, 1], F32, tag="epsn")
            nc.gpsimd.memset(eps_n[:], 1e-24)

            with (
                tc.tile_pool(name="xpool", bufs=3) as xpool,
                tc.tile_pool(name="lnsq", bufs=2) as lnsqp,
                tc.tile_pool(name="yTc", bufs=2) as yTp,
                tc.tile_pool(name="qkTc", bufs=4) as qkTp,
                tc.tile_pool(name="vc", bufs=4) as vsbp,
                tc.tile_pool(name="oTc", bufs=4) as oTp,
                tc.tile_pool(name="stat", bufs=6) as statp,
                tc.tile_pool(name="rows", bufs=6) as rowsp,
                tc.tile_pool(name="bc", bufs=4) as bcp,
                tc.tile_pool(name="expool", bufs=4) as exp_p,
                tc.tile_pool(name="otmp", bufs=2) as otmpp,
                tc.tile_pool(name="rdp", bufs=2) as rdp,
                tc.tile_pool(name="projp", bufs=3) as projp,
            ):
                qkT_c = []     # per chunk: [128, 4*512], col = dm*512 + t
                v_c = []       # per chunk: [128, 4*HPC*VW]
                oT_c = []      # per chunk: [128, 2*512], col = kt2*512 + t

                def ln_chunk(c4):
                    yTt = yTp.tile([128, KD * 512], BF, tag="yT")
                    for mm in range(4):
                        m = 4 * c4 + mm
                        xt = xpool.tile([128, DIM], F32, tag="x")
                        nc.sync.dma_start(
                            xt[:], xb.ap()[m * 128 : (m + 1) * 128, :]
                        )
                        st = statp.tile([128, 8], F32, tag="st")
                        nc.vector.reduce_sum(st[:, 0:1], xt[:], axis=AX.X)
                        sq = lnsqp.tile([128, DIM], F32, tag="lnsq")
                        nc.scalar.activation(
                            sq[:], xt[:], AF.Square, accum_out=st[:, 1:2]
                        )
                        nc.vector.tensor_scalar_mul(
                            st[:, 2:3], st[:, 0:1], 1.0 / DIM
                        )
                        nc.vector.tensor_tensor(
                            st[:, 3:4], st[:, 2:3], st[:, 2:3], op=OP.mult
                        )
                        nc.vector.scalar_tensor_tensor(
                            st[:, 5:6], st[:, 1:2], 1.0 / DIM, st[:, 3:4],
                            op0=OP.mult, op1=OP.subtract,
                        )
                        nc.scalar.activation(
                            st[:, 6:7], st[:, 5:6], AF.Sqrt, bias=eps_ln[:]
                        )
                        nc.vector.reciprocal_approx_fast(st[:, 7:8], st[:, 6:7])
                        xb16 = xpool.tile([128, DIM], BF, tag="xb16")
                        nc.vector.tensor_scalar(
                            xb16[:], xt[:], st[:, 2:3], st[:, 7:8],
                            op0=OP.subtract, op1=OP.mult,
                        )
                        nc.sync.dma_start_transpose(
                            yTt[:]
                            .rearrange("p (kt t) -> p kt t", kt=KD)[
                                :, :, mm * 128 : (mm + 1) * 128
                            ],
                            xb16[:],
                        )
                    return yTt

                def qkv_chunk(c4, yTt):
                    qkTt = qkTp.tile([128, 4 * 512], BF, tag="qkT")
                    qkT_c.append(qkTt)
                    for dm in range(4):
                        ps = pspool.tile([128, 512], F32, tag="mm")
                        for kt in range(KD):
                            nc.tensor.matmul(
                                ps[:],
                                w_qk[:, kt * 512 + dm * 128 : kt * 512 + (dm + 1) * 128],
                                yTt[:, kt * 512 : (kt + 1) * 512],
                                start=(kt == 0),
                                stop=(kt == KD - 1),
                            )
                        nc.scalar.activation(
                            qkTt[:, dm * 512 : (dm + 1) * 512],
                            ps[:],
                            AF.Identity,
                            bias=b_qk[:, dm : dm + 1],
                        )
                    vt = vsbp.tile([128, 4 * HPC * VW], BF, tag="v")
                    v_c.append(vt)
                    nc.gpsimd.memset(vt[:], 1.0)
                    for mm in range(4):
                        psv = pspool.tile([128, HD], F32, tag="mm")
                        for kt in range(KD):
                            nc.tensor.matmul(
                                psv[:],
                                yTt[:, kt * 512 + mm * 128 : kt * 512 + (mm + 1) * 128],
                                w_v[:, kt * HD : (kt + 1) * HD],
                                start=(kt == 0),
                                stop=(kt == KD - 1),
                            )
                        vdst = vt[:].rearrange(
                            "p (mt h e) -> p mt h e", mt=4, h=HPC
                        )[:, mm, :, 0:DH]
                        nc.scalar.copy(
                            vdst, psv[:].rearrange("p (h e) -> p h e", h=HPC)
                        )

                def norms_chunk(c4):
                    qkTt = qkT_c[c4]
                    for grp in range(4):
                        sqt = lnsqp.tile([128, 512], BF, tag="lnsqb")
                        nc.scalar.activation(
                            sqt[:], qkTt[:, grp * 512 : (grp + 1) * 512], AF.Square
                        )
                        psn = pspool.tile([2, 512], F32, tag="mm")
                        nc.tensor.matmul(
                            psn[:], hsel_sb[:], sqt[:], start=True, stop=True
                        )
                        srow = rowsp.tile([2, 512], F32, tag="srow")
                        nc.scalar.activation(
                            srow[:], psn[:], AF.Sqrt, bias=eps_n[0:2, :]
                        )
                        nc.vector.reciprocal_approx_fast(srow[:], srow[:])
                        for r in range(2):
                            prow = r * 64
                            src = srow[0:1, :]
                            if r == 1:
                                srow2 = rowsp.tile([2, 512], F32, tag="srow")
                                nc.sync.dma_start(srow2[0:1, :], srow[1:2, :])
                                src = srow2[0:1, :]
                            bc = bcp.tile([128, 512], F32, tag="bc")
                            nc.gpsimd.partition_broadcast(bc[:], src)
                            sl = qkTt[
                                prow : prow + 64, grp * 512 : (grp + 1) * 512
                            ]
                            nc.vector.tensor_tensor(
                                sl, sl, bc[prow : prow + 64, :], op=OP.mult
                            )

                def attn_head(qc, h):
                    prow = (h % 2) * 64
                    qkq = qkT_c[qc]
                    pso = opspool.tile([65, 512], F32, tag="o")
                    nkt = 4 * qc + 4
                    for pg in range(nkt // 2):
                        pair = tuple(
                            (2 * pg + i, max(0, 2 * pg + i - 4 * qc) * 128)
                            for i in range(2)
                        )
                        pss = simpool.tile([128, 1024], F32, tag="sim")
                        for i, (kt, of) in enumerate(pair):
                            nc.tensor.matmul(
                                pss[:, i * 512 + of : (i + 1) * 512],
                                qkT_c[kt // 4][
                                    prow : prow + 64,
                                    (2 + h // 2) * 512 + (kt % 4) * 128 :
                                    (2 + h // 2) * 512 + (kt % 4 + 1) * 128,
                                ],
                                qkq[
                                    prow : prow + 64,
                                    (h // 2) * 512 + of : (h // 2 + 1) * 512,
                                ],
                                start=True,
                                stop=True,
                            )
                        of0 = pair[0][1]
                        ex = exp_p.tile([128, 1024], BF, tag="ex")
                        nc.scalar.activation(
                            ex[:, of0:1024], pss[:, of0:1024], AF.Exp, scale=SCALE
                        )
                        for i, (kt, of) in enumerate(pair):
                            if kt >= 4 * qc:
                                db = i * 512 + of
                                nc.vector.tensor_tensor(
                                    ex[:, db : db + 128],
                                    ex[:, db : db + 128],
                                    mask_sb[:],
                                    op=OP.mult,
                                )
                            nc.tensor.matmul(
                                pso[:, of:512],
                                v_c[kt // 4][
                                    :,
                                    ((kt % 4) * HPC + h) * VW :
                                    ((kt % 4) * HPC + h) * VW + VW,
                                ],
                                ex[:, i * 512 + of : (i + 1) * 512],
                                start=(kt == 0),
                                stop=(kt == nkt - 1),
                            )
                    rdt = rdp.tile([65, 512], F32, tag="rd")
                    nc.scalar.copy(rdt[64:65, :], pso[64:65, :])
                    rd0 = rowsp.tile([2, 512], F32, tag="srow")
                    nc.sync.dma_start(rd0[0:1, :], rdt[64:65, :])
                    nc.vector.reciprocal_approx_fast(rd0[0:1, :], rd0[0:1, :])
                    bc = bcp.tile([128, 512], F32, tag="bc")
                    nc.gpsimd.partition_broadcast(bc[:], rd0[0:1, :])
                    oTt = oT_c[qc]
                    if prow == 0:
                        nc.vector.tensor_tensor(
                            oTt[0:64, (h // 2) * 512 : (h // 2 + 1) * 512],
                            pso[0:64, :],
                            bc[0:64, :],
                            op=OP.mult,
                        )
                    else:
                        ot = otmpp.tile([64, 512], BF, tag="otmp")
                        nc.vector.tensor_tensor(
                            ot[:], pso[0:64, :], bc[0:64, :], op=OP.mult
                        )
                        nc.sync.dma_start(
                            oTt[64:128, (h // 2) * 512 : (h // 2 + 1) * 512],
                            ot[:],
                        )

                def proj_chunk(qc):
                    oTt = oT_c[qc]
                    for mm in range(4):
                        pj = projp.tile([128, DIM], BF, tag="proj")
                        for cc in range(2):
                            psp = pspool.tile([128, 512], F32, tag="mm")
                            for kt2 in range(2):
                                nc.tensor.matmul(
                                    psp[:],
                                    oTt[:, kt2 * 512 + mm * 128 : kt2 * 512 + (mm + 1) * 128],
                                    w_p[:, kt2 * DIM + cc * 512 : kt2 * DIM + (cc + 1) * 512],
                                    start=(kt2 == 0),
                                    stop=(kt2 == 1),
                                )
                            nc.scalar.copy(pj[:, cc * 512 : (cc + 1) * 512], psp[:])
                        nc.sync.dma_start(
                            proj_c[qc].ap()[mm * 128 : (mm + 1) * 128, :], pj[:]
                        )
                    nc.gpsimd.collective_compute(
                        "ReduceScatter",
                        OP.add,
                        replica_groups=GROUPS,
                        ins=[proj_c[qc].ap().opt()],
                        outs=[rs_c[qc].ap().opt()],
                    )

                for c4 in range(4):
                    yTt = ln_chunk(c4)
                    qkv_chunk(c4, yTt)
                    norms_chunk(c4)
                    oT_c.append(
                        oTp.tile([128, 2 * 512], BF, tag="oT", name=f"oT{c4}")
                    )
                    for h in range(HPC):
                        attn_head(c4, h)
                    proj_chunk(c4)

                if DEBUG_DUMPS:
                    for c4 in range(4):
                        nc.sync.dma_start(
                            d_qkT.ap().rearrange("p (dm t) -> p dm t", dm=4)[
                                :, :, c4 * 512 : (c4 + 1) * 512
                            ],
                            qkT_c[c4][:].rearrange("p (dm t) -> p dm t", dm=4),
                        )
                        nc.sync.dma_start(
                            d_v.ap().rearrange("p (c r) -> p c r", c=4)[:, c4, :],
                            v_c[c4][:],
                        )
                        nc.sync.dma_start(
                            d_oT.ap().rearrange("p (k t) -> p k t", k=2)[
                                :, :, c4 * 512 : (c4 + 1) * 512
                            ],
                            oT_c[c4][:].rearrange("p (k t) -> p k t", k=2),
                        )

            # ---- to_out on the token shard, two 256-token batches ----
            with tc.tile_pool(name="tail", bufs=1) as tailp, tc.tile_pool(
                name="tout", bufs=2
            ) as toutp:
                w_to = tailp.tile([128, KD * DIM], BF, tag="wto")
                nc.sync.dma_start(
                    w_to[:].rearrange("p (kt m) -> p kt m", kt=KD),
                    wto.ap().rearrange("(kt p) m -> p kt m", p=128),
                )
                for half in range(2):
                    rsT = tailp.tile([128, KD * 256], BF, tag=f"rsT{half}")
                    for si in range(2):
                        s = 2 * half + si
                        rst = tailp.tile([128, DIM], BF, tag=f"rst{half}{si}")
                        nc.sync.dma_start(rst[:], rs_c[s].ap())
                        nc.sync.dma_start_transpose(
                            rsT[:]
                            .rearrange("p (kt t) -> p kt t", kt=KD)[
                                :, :, si * 128 : (si + 1) * 128
                            ],
                            rst[:],
                        )
                    for jm in range(KD):
                        pst = pspool.tile([128, 512], F32, tag="mm")
                        for kt in range(KD):
                            nc.tensor.matmul(
                                pst[:, 0:256],
                                w_to[:, kt * DIM + jm * 128 : kt * DIM + (jm + 1) * 128],
                                rsT[:, kt * 256 : (kt + 1) * 256],
                                start=(kt == 0),
                                stop=(kt == KD - 1),
                            )
                        ot = toutp.tile([128, 256], F32, tag="ot")
                        nc.scalar.activation(
                            ot[:], pst[:, 0:256], AF.Identity,
                            bias=b_out[:, jm : jm + 1],
                        )
                        nc.sync.dma_start(
                            out_ext.ap()[
                                jm * 128 : (jm + 1) * 128,
                                half * 256 : (half + 1) * 256,
                            ],
                            ot[:],
                        )

    nc.compile()
    return nc


_NC = None


def _get_nc():
    global _NC
    if _NC is None:
        _NC = _build_nc()
    return _NC


def _prep_inputs(x, g, b, c_attn_w, c_attn_b, c_proj_w, c_proj_b, to_out_w):
    """Host-side fold + shard.  Returns per-core input maps."""
    f32 = np.float32
    bf16 = ml_dtypes.bfloat16
    W = (g[:, None] * c_attn_w).astype(f32)          # [DIM, 3*DIM]
    cb = (c_attn_b + b @ c_attn_w).astype(f32)       # [3*DIM]
    cb_v = cb[2 * DIM :]
    b_out = ((c_proj_b + cb_v @ c_proj_w) @ to_out_w.T).astype(f32)
    w_to_T = np.ascontiguousarray(to_out_w.T).astype(bf16)
    mask = np.triu(np.ones((128, 128), dtype=f32))   # allow q >= k in [k,q] layout
    hs2 = np.zeros((128, 2), dtype=bf16)
    hs2[:64, 0] = 1.0
    hs2[64:, 1] = 1.0

    in_maps = []
    for c in range(N_CORES):
        bi, gi = c // 4, c % 4
        h0 = gi * HPC * DH                           # 256*gi
        w_qk = np.ascontiguousarray(
            np.concatenate(
                [W[:, h0 : h0 + HD], W[:, DIM + h0 : DIM + h0 + HD]], axis=1
            )
        ).astype(bf16)
        b_qk = np.ascontiguousarray(
            np.concatenate([cb[h0 : h0 + HD], cb[DIM + h0 : DIM + h0 + HD]]),
            dtype=f32,
        )
        w_v = np.ascontiguousarray(W[:, 2 * DIM + h0 : 2 * DIM + h0 + HD]).astype(bf16)
        w_p = np.ascontiguousarray(c_proj_w[h0 : h0 + HD, :]).astype(bf16)
        in_maps.append(
            {
                "xb": np.ascontiguousarray(x[bi], dtype=f32),
                "wqk": w_qk,
                "bqk": b_qk,
                "wv": w_v,
                "wp": w_p,
                "wto": w_to_T,
                "bout": b_out,
                "maskd": mask,
                "hsel": hs2,
            }
        )
    return in_maps


def kernel(x, g, b, c_attn_w, c_attn_b, c_proj_w, c_proj_b, to_out_w, **kw):
    nc = _get_nc()
    in_maps = _prep_inputs(
        np.asarray(x), np.asarray(g), np.asarray(b), np.asarray(c_attn_w),
        np.asarray(c_attn_b), np.asarray(c_proj_w), np.asarray(c_proj_b),
        np.asarray(to_out_w),
    )
    res = run_bass_kernel_spmd(nc, in_maps, list(range(N_CORES)), **kw)
    out = np.empty((B, N, DIM), dtype=np.float32)
    for c in range(N_CORES):
        bi, gi = c // 4, c % 4
        o = res.results[c]["out"]                    # [DIM, NSH], strips of 128
        for s in range(4):
            out[bi, s * 512 + gi * 128 : s * 512 + (gi + 1) * 128, :] = (
                o[:, s * 128 : (s + 1) * 128].T
            )
    kernel.last_result = res
    return out


# revision 33
# speedup vs baseline: 1.1931x; 1.1931x over previous
"""Trainium2 Bass kernel for a cosine-sim causal attention block (8 NeuronCores).

Reference computation (single device):
  y   = LayerNorm(x) * g + b
  qkv = y @ c_attn_w + c_attn_b ; split q,k,v into 16 heads of 64
  qn, kn = l2norm(q), l2norm(k)
  attn = softmax(causal(8 * qn @ kn^T))
  o    = attn @ v  (merge heads)
  o    = o @ c_proj_w + c_proj_b
  out  = o @ to_out_w.T

Sharding: 2 batch groups x 4 cores; core owns one batch and 4 heads
(tensor-parallel c_attn columns / c_proj rows).  Chunked ReduceScatter of the
c_proj partials inside each group token-shards the output; each core then
runs to_out on its 512-token shard.  Host folds g/b into c_attn, the v-bias
and c_proj bias through to a single to_out bias, and reassembles shards.

Compute: bf16 matmul operands, fp32 accumulation/softmax statistics.
Attention runs in the transposed [k, q] layout so softmax + attn@v need no
on-chip transposes; softmax denominators come free from a ones-column
appended to v; q/k norms fold into broadcast multiplies; the causal mask is
added onto the sim PSUM by a small PE matmul (exp then underflows to zero).
"""

import sys

for _p in ("/opt/trn_rl_repo", "/root/.axon_site/_ro/trn_rl_repo"):
    if _p not in sys.path:
        sys.path.insert(0, _p)

import numpy as np
import ml_dtypes

from concourse import bass, bacc, tile, mybir
from concourse.bass_utils import run_bass_kernel_spmd

B, N, DIM, H, DH = 2, 2048, 1024, 16, 64
SCALE = 8.0
LN_EPS = 1e-5

N_CORES = 8
GROUPS = [[0, 1, 2, 3], [4, 5, 6, 7]]
HPC = 4            # heads per core
HD = HPC * DH      # 256 head dims per core
NSH = N // 4       # 512 tokens per core after reduce-scatter

F32 = mybir.dt.float32
BF = mybir.dt.bfloat16
AF = mybir.ActivationFunctionType
OP = mybir.AluOpType
AX = mybir.AxisListType

NT = N // 128      # 16 token tiles
KD = DIM // 128    # 8 dim tiles
VW = DH + 1        # per-head v block width incl ones column

DEBUG_DUMPS = False


def _build_nc():
    nc = bacc.Bacc(
        "TRN2", target_bir_lowering=False, debug=False, num_devices=N_CORES
    )

    xb = nc.dram_tensor("xb", [N, DIM], F32, kind="ExternalInput")
    wqk = nc.dram_tensor("wqk", [DIM, 512], BF, kind="ExternalInput")
    bqk = nc.dram_tensor("bqk", [512], F32, kind="ExternalInput")
    wv = nc.dram_tensor("wv", [DIM, HD], BF, kind="ExternalInput")
    wp = nc.dram_tensor("wp", [HD, DIM], BF, kind="ExternalInput")
    wto = nc.dram_tensor("wto", [DIM, DIM], BF, kind="ExternalInput")
    bout = nc.dram_tensor("bout", [DIM], F32, kind="ExternalInput")
    maskT = nc.dram_tensor("maskT", [128, 128], BF, kind="ExternalInput")
    idb = nc.dram_tensor("idb", [128, 128], BF, kind="ExternalInput")
    hsel = nc.dram_tensor("hsel", [128, 2], BF, kind="ExternalInput")
    ident = nc.dram_tensor("ident", [128, 128], F32, kind="ExternalInput")
    out_ext = nc.dram_tensor("out", [DIM, NSH], F32, kind="ExternalOutput")

    proj_c = [nc.dram_tensor(f"proj_b{c}", [NSH, DIM], BF) for c in range(4)]
    rs_c = [nc.dram_tensor(f"rs_b{c}", [128, DIM], BF) for c in range(4)]
    if DEBUG_DUMPS:
        d_qkT = nc.dram_tensor("d_qkT", [128, 4 * N], BF, kind="ExternalOutput")
        d_v = nc.dram_tensor("d_v", [128, NT * HPC * VW], BF, kind="ExternalOutput")
        d_oT = nc.dram_tensor("d_oT", [128, 2 * N], BF, kind="ExternalOutput")

    with tile.TileContext(nc) as tc:
        with (
            tc.tile_pool(name="wpool", bufs=1) as wpool,
            tc.tile_pool(name="small", bufs=1) as small,
            tc.tile_pool(name="ps", bufs=2, space="PSUM") as pspool,
            tc.tile_pool(name="pss", bufs=2, space="PSUM") as simpool,
            tc.tile_pool(name="pso", bufs=2, space="PSUM") as opspool,
        ):
            # ---- persistent weights ----
            w_qk = wpool.tile([128, KD * 512], BF, tag="wqk")
            nc.sync.dma_start(
                w_qk[:].rearrange("p (kt m) -> p kt m", kt=KD),
                wqk.ap().rearrange("(kt p) m -> p kt m", p=128),
            )
            w_v = wpool.tile([128, KD * HD], BF, tag="wv")
            nc.sync.dma_start(
                w_v[:].rearrange("p (kt m) -> p kt m", kt=KD),
                wv.ap().rearrange("(kt p) m -> p kt m", p=128),
            )
            w_p = wpool.tile([128, 2 * DIM], BF, tag="wp")
            nc.sync.dma_start(
                w_p[:].rearrange("p (kt m) -> p kt m", kt=2),
                wp.ap().rearrange("(kt p) m -> p kt m", p=128),
            )
            b_qk = small.tile([128, 4], F32, tag="bqk")
            nc.sync.dma_start(b_qk[:], bqk.ap().rearrange("(mt p) -> p mt", p=128))
            b_out = small.tile([128, 8], F32, tag="bout")
            nc.sync.dma_start(b_out[:], bout.ap().rearrange("(mt p) -> p mt", p=128))
            maskT_sb = small.tile([128, 128], BF, tag="maskT")
            nc.sync.dma_start(maskT_sb[:], maskT.ap())
            idb_sb = small.tile([128, 128], BF, tag="idb")
            nc.sync.dma_start(idb_sb[:], idb.ap())
            hsel_sb = small.tile([128, 2], BF, tag="hsel")
            nc.sync.dma_start(hsel_sb[:], hsel.ap())
            id_sb = small.tile([128, 128], F32, tag="ident")
            nc.sync.dma_start(id_sb[:], ident.ap())

            eps_ln = small.tile([128, 1], F32, tag="epsln")
            nc.gpsimd.memset(eps_ln[:], LN_EPS)
            eps_n = small.tile([128, 1], F32, tag="epsn")
            nc.gpsimd.memset(eps_n[:], 1e-24)

            with (
                tc.tile_pool(name="xpool", bufs=16) as xpool,
                tc.tile_pool(name="lnsq", bufs=2) as lnsqp,
                tc.tile_pool(name="yT", bufs=1) as yTp,
                tc.tile_pool(name="qkT", bufs=1) as qkTp,
                tc.tile_pool(name="vsb", bufs=1) as vsbp,
                tc.tile_pool(name="stat", bufs=1) as statp,
                tc.tile_pool(name="rows", bufs=6) as rowsp,
                tc.tile_pool(name="bc", bufs=4) as bcp,
            ):
                yT = yTp.tile([128, KD * N], BF, tag="yT")
                qkT = qkTp.tile([128, 4 * N], BF, tag="qkT")
                v_sb = vsbp.tile([128, NT * HPC * VW], BF, tag="v")
                nc.gpsimd.memset(v_sb[:], 1.0)

                # ---- LayerNorm: batched stats, then normalize + transpose ----
                sums = statp.tile([128, 16], F32, tag="sums")
                sumsq = statp.tile([128, 16], F32, tag="sumsq")
                mu = statp.tile([128, 16], F32, tag="mu")
                musq = statp.tile([128, 16], F32, tag="musq")
                var = statp.tile([128, 16], F32, tag="var")
                rstd = statp.tile([128, 16], F32, tag="rstd")
                xts = []
                for m in range(NT):
                    xt = xpool.tile([128, DIM], F32, tag="x", name=f"x{m}")
                    xts.append(xt)
                    nc.sync.dma_start(xt[:], xb.ap()[m * 128 : (m + 1) * 128, :])
                    nc.vector.reduce_sum(sums[:, m : m + 1], xt[:], axis=AX.X)
                    sq = lnsqp.tile([128, DIM], F32, tag="lnsq")
                    nc.scalar.activation(
                        sq[:], xt[:], AF.Square, accum_out=sumsq[:, m : m + 1]
                    )
                nc.vector.tensor_scalar_mul(mu[:], sums[:], 1.0 / DIM)
                nc.vector.tensor_tensor(musq[:], mu[:], mu[:], op=OP.mult)
                nc.vector.scalar_tensor_tensor(
                    var[:], sumsq[:], 1.0 / DIM, musq[:],
                    op0=OP.mult, op1=OP.subtract,
                )
                nc.scalar.activation(var[:], var[:], AF.Sqrt, bias=eps_ln[:])
                nc.vector.reciprocal_approx_fast(rstd[:], var[:])
                for m in range(NT):
                    nc.vector.tensor_scalar(
                        xts[m][:], xts[m][:], mu[:, m : m + 1], rstd[:, m : m + 1],
                        op0=OP.subtract, op1=OP.mult,
                    )
                # transpose y -> yT (PE), 4 token tiles per PSUM bank
                for kt in range(KD):
                    for mg in range(4):
                        pst = pspool.tile([128, 512], F32, tag="mm")
                        for j in range(4):
                            m = mg * 4 + j
                            nc.tensor.transpose(
                                pst[:, j * 128 : (j + 1) * 128],
                                xts[m][:, kt * 128 : (kt + 1) * 128],
                                id_sb[:],
                            )
                        nc.scalar.copy(
                            yT[:, kt * N + mg * 512 : kt * N + (mg + 1) * 512],
                            pst[:],
                        )

                # ---- qk^T = W_qk^T @ y^T ----
                for c4 in range(4):
                    for dm in range(4):
                        ps = pspool.tile([128, 512], F32, tag="mm")
                        for kt in range(KD):
                            nc.tensor.matmul(
                                ps[:],
                                w_qk[:, kt * 512 + dm * 128 : kt * 512 + (dm + 1) * 128],
                                yT[:, kt * N + c4 * 512 : kt * N + (c4 + 1) * 512],
                                start=(kt == 0),
                                stop=(kt == KD - 1),
                            )
                        nc.scalar.activation(
                            qkT[:, dm * N + c4 * 512 : dm * N + (c4 + 1) * 512],
                            ps[:],
                            AF.Identity,
                            bias=b_qk[:, dm : dm + 1],
                        )

                # ---- v natural = y @ W_v, interleaved with ones columns ----
                for m in range(NT):
                    psv = pspool.tile([128, HD], F32, tag="mm")
                    for kt in range(KD):
                        nc.tensor.matmul(
                            psv[:],
                            yT[:, kt * N + m * 128 : kt * N + (m + 1) * 128],
                            w_v[:, kt * HD : (kt + 1) * HD],
                            start=(kt == 0),
                            stop=(kt == KD - 1),
                        )
                    vdst = v_sb[:].rearrange(
                        "p (mt h e) -> p mt h e", mt=NT, h=HPC
                    )[:, m, :, 0:DH]
                    nc.scalar.copy(
                        vdst, psv[:].rearrange("p (h e) -> p h e", h=HPC)
                    )

                # ---- L2 norms; normalize q AND k in place (bf16) ----
                for grp in range(4):
                    for c4 in range(4):
                        sqt = lnsqp.tile([128, 512], BF, tag="lnsqb")
                        nc.scalar.activation(
                            sqt[:],
                            qkT[:, grp * N + c4 * 512 : grp * N + (c4 + 1) * 512],
                            AF.Square,
                        )
                        psn = pspool.tile([2, 512], F32, tag="mm")
                        nc.tensor.matmul(
                            psn[:], hsel_sb[:], sqt[:], start=True, stop=True
                        )
                        srow = rowsp.tile([2, 512], F32, tag="srow")
                        nc.scalar.activation(
                            srow[:], psn[:], AF.Sqrt, bias=eps_n[0:2, :]
                        )
                        nc.vector.reciprocal_approx_fast(srow[:], srow[:])
                        for r in range(2):
                            prow = r * 64
                            src = srow[0:1, :]
                            if r == 1:
                                srow2 = rowsp.tile([2, 512], F32, tag="srow")
                                nc.sync.dma_start(srow2[0:1, :], srow[1:2, :])
                                src = srow2[0:1, :]
                            bc = bcp.tile([128, 512], F32, tag="bc")
                            nc.gpsimd.partition_broadcast(bc[:], src)
                            sl = qkT[
                                prow : prow + 64,
                                grp * N + c4 * 512 : grp * N + (c4 + 1) * 512,
                            ]
                            nc.vector.tensor_tensor(
                                sl, sl, bc[prow : prow + 64, :], op=OP.mult
                            )

                # ---- attention + c_proj + chunked reduce-scatter ----
                oT = qkTp.tile([128, 2 * N], BF, tag="oT")

                def attn_head(qc, h, exp_p, otmpp, rdp):
                    prow = (h % 2) * 64
                    qoff = (h // 2) * N
                    koff = (2 + h // 2) * N
                    pso = opspool.tile([65, 512], F32, tag="o")
                    nkt = 4 * qc + 4
                    for pg in range(nkt // 2):
                        pair = tuple(
                            (2 * pg + i, max(0, 2 * pg + i - 4 * qc) * 128)
                            for i in range(2)
                        )
                        pss = simpool.tile([128, 1024], F32, tag="sim")
                        for i, (kt, of) in enumerate(pair):
                            diag = kt >= 4 * qc
                            nc.tensor.matmul(
                                pss[:, i * 512 + of : (i + 1) * 512],
                                qkT[
                                    prow : prow + 64,
                                    koff + kt * 128 : koff + (kt + 1) * 128,
                                ],
                                qkT[
                                    prow : prow + 64,
                                    qoff + qc * 512 + of : qoff + (qc + 1) * 512,
                                ],
                                start=True,
                                stop=not diag,
                            )
                            if diag:
                                # sim += maskAdd on the diagonal block: exp of
                                # masked entries underflows to zero
                                db = i * 512 + of
                                nc.tensor.matmul(
                                    pss[:, db : db + 128],
                                    maskT_sb[:],
                                    idb_sb[:],
                                    start=False,
                                    stop=True,
                                )
                        of0 = pair[0][1]
                        ex = exp_p.tile([128, 1024], BF, tag="ex")
                        nc.scalar.activation(
                            ex[:, of0:1024], pss[:, of0:1024], AF.Exp, scale=SCALE
                        )
                        for i, (kt, of) in enumerate(pair):
                            nc.tensor.matmul(
                                pso[:, of:512],
                                v_sb[
                                    :,
                                    kt * HPC * VW + h * VW :
                                    kt * HPC * VW + h * VW + VW,
                                ],
                                ex[:, i * 512 + of : (i + 1) * 512],
                                start=(kt == 0),
                                stop=(kt == nkt - 1),
                            )
                    rdt = rdp.tile([65, 512], F32, tag="rd")
                    nc.scalar.copy(rdt[64:65, :], pso[64:65, :])
                    rd0 = rowsp.tile([2, 512], F32, tag="srow")
                    nc.sync.dma_start(rd0[0:1, :], rdt[64:65, :])
                    nc.vector.reciprocal_approx_fast(rd0[0:1, :], rd0[0:1, :])
                    bc = bcp.tile([128, 512], F32, tag="bc")
                    nc.gpsimd.partition_broadcast(bc[:], rd0[0:1, :])
                    if prow == 0:
                        nc.vector.tensor_tensor(
                            oT[0:64, qoff + qc * 512 : qoff + (qc + 1) * 512],
                            pso[0:64, :],
                            bc[0:64, :],
                            op=OP.mult,
                        )
                    else:
                        ot = otmpp.tile([64, 512], BF, tag="otmp")
                        nc.vector.tensor_tensor(
                            ot[:], pso[0:64, :], bc[0:64, :], op=OP.mult
                        )
                        nc.sync.dma_start(
                            oT[64:128, qoff + qc * 512 : qoff + (qc + 1) * 512],
                            ot[:],
                        )

                def proj_chunk(qc, projp):
                    for mm in range(4):
                        m = 4 * qc + mm
                        pj = projp.tile([128, DIM], BF, tag="proj")
                        for cc in range(2):
                            psp = pspool.tile([128, 512], F32, tag="mm")
                            for kt2 in range(2):
                                nc.tensor.matmul(
                                    psp[:],
                                    oT[:, kt2 * N + m * 128 : kt2 * N + (m + 1) * 128],
                                    w_p[:, kt2 * DIM + cc * 512 : kt2 * DIM + (cc + 1) * 512],
                                    start=(kt2 == 0),
                                    stop=(kt2 == 1),
                                )
                            nc.scalar.copy(pj[:, cc * 512 : (cc + 1) * 512], psp[:])
                        nc.sync.dma_start(
                            proj_c[qc].ap()[mm * 128 : (mm + 1) * 128, :], pj[:]
                        )
                    nc.gpsimd.collective_compute(
                        "ReduceScatter",
                        OP.add,
                        replica_groups=GROUPS,
                        ins=[proj_c[qc].ap().opt()],
                        outs=[rs_c[qc].ap().opt()],
                    )

                with tc.tile_pool(name="expool", bufs=4) as exp_p, tc.tile_pool(
                    name="otmp", bufs=2
                ) as otmpp, tc.tile_pool(name="rdp", bufs=2) as rdp, tc.tile_pool(
                    name="projp", bufs=3
                ) as projp:
                    for qc in range(4):
                        for h in range(HPC):
                            attn_head(qc, h, exp_p, otmpp, rdp)
                        proj_chunk(qc, projp)

                if DEBUG_DUMPS:
                    nc.sync.dma_start(d_qkT.ap(), qkT[:])
                    nc.sync.dma_start(d_v.ap(), v_sb[:])
                    nc.sync.dma_start(d_oT.ap(), oT[:])

            # ---- to_out on the token shard, two 256-token batches ----
            with tc.tile_pool(name="tail", bufs=1) as tailp, tc.tile_pool(
                name="tout", bufs=2
            ) as toutp:
                w_to = tailp.tile([128, KD * DIM], BF, tag="wto")
                nc.sync.dma_start(
                    w_to[:].rearrange("p (kt m) -> p kt m", kt=KD),
                    wto.ap().rearrange("(kt p) m -> p kt m", p=128),
                )
                for half in range(2):
                    rsT = tailp.tile(
                        [128, KD * 256], BF, tag=f"rsT{half}", name=f"rsT{half}"
                    )
                    for si in range(2):
                        s = 2 * half + si
                        rst = tailp.tile(
                            [128, DIM], BF, tag=f"rst{half}{si}", name=f"rst{s}"
                        )
                        nc.sync.dma_start(rst[:], rs_c[s].ap())
                        nc.sync.dma_start_transpose(
                            rsT[:]
                            .rearrange("p (kt t) -> p kt t", kt=KD)[
                                :, :, si * 128 : (si + 1) * 128
                            ],
                            rst[:],
                        )
                    for jm in range(KD):
                        pst = pspool.tile([128, 512], F32, tag="mm")
                        for kt in range(KD):
                            nc.tensor.matmul(
                                pst[:, 0:256],
                                w_to[:, kt * DIM + jm * 128 : kt * DIM + (jm + 1) * 128],
                                rsT[:, kt * 256 : (kt + 1) * 256],
                                start=(kt == 0),
                                stop=(kt == KD - 1),
                            )
                        ot = toutp.tile([128, 256], F32, tag="ot")
                        nc.scalar.activation(
                            ot[:], pst[:, 0:256], AF.Identity,
                            bias=b_out[:, jm : jm + 1],
                        )
                        nc.sync.dma_start(
                            out_ext.ap()[
                                jm * 128 : (jm + 1) * 128,
                                half * 256 : (half + 1) * 256,
                            ],
                            ot[:],
                        )

    nc.compile()
    return nc


_NC = None


def _get_nc():
    global _NC
    if _NC is None:
        _NC = _build_nc()
    return _NC


def _prep_inputs(x, g, b, c_attn_w, c_attn_b, c_proj_w, c_proj_b, to_out_w):
    """Host-side fold + shard.  Returns per-core input maps."""
    f32 = np.float32
    bf16 = ml_dtypes.bfloat16
    W = (g[:, None] * c_attn_w).astype(f32)          # [DIM, 3*DIM]
    cb = (c_attn_b + b @ c_attn_w).astype(f32)       # [3*DIM]
    cb_v = cb[2 * DIM :]
    b_out = ((c_proj_b + cb_v @ c_proj_w) @ to_out_w.T).astype(f32)
    w_to_T = np.ascontiguousarray(to_out_w.T).astype(bf16)
    # mask added onto sim in [k, q] layout: -100 where q < k (strictly lower)
    mask_add = (-100.0 * (1.0 - np.triu(np.ones((128, 128))))).astype(f32)
    maskT = np.ascontiguousarray(mask_add.T).astype(bf16)
    idb = np.eye(128).astype(bf16)
    hs2 = np.zeros((128, 2), dtype=bf16)
    hs2[:64, 0] = 1.0
    hs2[64:, 1] = 1.0
    ident = np.eye(128, dtype=f32)

    in_maps = []
    for c in range(N_CORES):
        bi, gi = c // 4, c % 4
        h0 = gi * HPC * DH                           # 256*gi
        w_qk = np.ascontiguousarray(
            np.concatenate(
                [W[:, h0 : h0 + HD], W[:, DIM + h0 : DIM + h0 + HD]], axis=1
            )
        ).astype(bf16)
        b_qk = np.ascontiguousarray(
            np.concatenate([cb[h0 : h0 + HD], cb[DIM + h0 : DIM + h0 + HD]]),
            dtype=f32,
        )
        w_v = np.ascontiguousarray(W[:, 2 * DIM + h0 : 2 * DIM + h0 + HD]).astype(bf16)
        w_p = np.ascontiguousarray(c_proj_w[h0 : h0 + HD, :]).astype(bf16)
        in_maps.append(
            {
                "xb": np.ascontiguousarray(x[bi], dtype=f32),
                "wqk": w_qk,
                "bqk": b_qk,
                "wv": w_v,
                "wp": w_p,
                "wto": w_to_T,
                "bout": b_out,
                "maskT": maskT,
                "idb": idb,
                "hsel": hs2,
                "ident": ident,
            }
        )
    return in_maps


def kernel(x, g, b, c_attn_w, c_attn_b, c_proj_w, c_proj_b, to_out_w, **kw):
    nc = _get_nc()
    in_maps = _prep_inputs(
        np.asarray(x), np.asarray(g), np.asarray(b), np.asarray(c_attn_w),
        np.asarray(c_attn_b), np.asarray(c_proj_w), np.asarray(c_proj_b),
        np.asarray(to_out_w),
    )
    res = run_bass_kernel_spmd(nc, in_maps, list(range(N_CORES)), **kw)
    out = np.empty((B, N, DIM), dtype=np.float32)
    for c in range(N_CORES):
        bi, gi = c // 4, c % 4
        o = res.results[c]["out"]                    # [DIM, NSH], strips of 128
        for s in range(4):
            out[bi, s * 512 + gi * 128 : s * 512 + (gi + 1) * 128, :] = (
                o[:, s * 128 : (s + 1) * 128].T
            )
    kernel.last_result = res
    return out


# revision 38
# speedup vs baseline: 1.2413x; 1.0404x over previous
"""Trainium2 Bass kernel for a cosine-sim causal attention block (8 NeuronCores).

Reference computation (single device):
  y   = LayerNorm(x) * g + b
  qkv = y @ c_attn_w + c_attn_b ; split q,k,v into 16 heads of 64
  qn, kn = l2norm(q), l2norm(k)
  attn = softmax(causal(8 * qn @ kn^T))
  o    = attn @ v  (merge heads)
  o    = o @ c_proj_w + c_proj_b
  out  = o @ to_out_w.T

Sharding: 2 batch groups x 4 cores; core owns one batch and 4 heads
(tensor-parallel c_attn columns / c_proj rows).  Chunked ReduceScatter of the
c_proj partials inside each group token-shards the output; each core then
runs to_out on its 512-token shard.  Host folds g/b into c_attn, the v-bias
and c_proj bias through to a single to_out bias, and reassembles shards.

Compute: bf16 matmul operands, fp32 accumulation/softmax statistics.
Attention runs in the transposed [k, q] layout so softmax + attn@v need no
on-chip transposes; softmax denominators come free from a ones-column
appended to v; q/k norms fold into broadcast multiplies; the causal mask is
added onto the sim PSUM by a small PE matmul (exp then underflows to zero).
"""

import sys

for _p in ("/opt/trn_rl_repo", "/root/.axon_site/_ro/trn_rl_repo"):
    if _p not in sys.path:
        sys.path.insert(0, _p)

import numpy as np
import ml_dtypes

from concourse import bass, bacc, tile, mybir
from concourse.bass_utils import run_bass_kernel_spmd

B, N, DIM, H, DH = 2, 2048, 1024, 16, 64
SCALE = 8.0
LN_EPS = 1e-5

N_CORES = 8
GROUPS = [[0, 1, 2, 3], [4, 5, 6, 7]]
HPC = 4            # heads per core
HD = HPC * DH      # 256 head dims per core
NSH = N // 4       # 512 tokens per core after reduce-scatter

F32 = mybir.dt.float32
BF = mybir.dt.bfloat16
AF = mybir.ActivationFunctionType
OP = mybir.AluOpType
AX = mybir.AxisListType

NT = N // 128      # 16 token tiles
KD = DIM // 128    # 8 dim tiles
VW = DH + 1        # per-head v block width incl ones column

DEBUG_DUMPS = False


def _build_nc():
    nc = bacc.Bacc(
        "TRN2", target_bir_lowering=False, debug=False, num_devices=N_CORES
    )

    xb = nc.dram_tensor("xb", [N, DIM], F32, kind="ExternalInput")
    wqk = nc.dram_tensor("wqk", [DIM, 512], BF, kind="ExternalInput")
    bqk = nc.dram_tensor("bqk", [512], F32, kind="ExternalInput")
    wv = nc.dram_tensor("wv", [DIM, HD], BF, kind="ExternalInput")
    wp = nc.dram_tensor("wp", [HD, DIM], BF, kind="ExternalInput")
    wto = nc.dram_tensor("wto", [DIM, DIM], BF, kind="ExternalInput")
    bout = nc.dram_tensor("bout", [DIM], F32, kind="ExternalInput")
    maskT = nc.dram_tensor("maskT", [128, 128], BF, kind="ExternalInput")
    idb = nc.dram_tensor("idb", [128, 128], BF, kind="ExternalInput")
    hsel = nc.dram_tensor("hsel", [128, 2], BF, kind="ExternalInput")
    ident = nc.dram_tensor("ident", [128, 128], F32, kind="ExternalInput")
    out_ext = nc.dram_tensor("out", [DIM, NSH], F32, kind="ExternalOutput")

    proj_c = [nc.dram_tensor(f"proj_b{c}", [NSH, DIM], BF) for c in range(4)]
    rs_c = [nc.dram_tensor(f"rs_b{c}", [128, DIM], BF) for c in range(4)]
    if DEBUG_DUMPS:
        d_qkT = nc.dram_tensor("d_qkT", [128, 4 * N], BF, kind="ExternalOutput")
        d_v = nc.dram_tensor("d_v", [128, NT * HPC * VW], BF, kind="ExternalOutput")
        d_oT = nc.dram_tensor("d_oT", [128, 2 * N], BF, kind="ExternalOutput")

    with tile.TileContext(nc) as tc:
        with (
            tc.tile_pool(name="wpool", bufs=1) as wpool,
            tc.tile_pool(name="small", bufs=1) as small,
        ):
            # ---- persistent weights ----
            w_qk = wpool.tile([128, KD * 512], BF, tag="wqk")
            nc.sync.dma_start(
                w_qk[:].rearrange("p (kt m) -> p kt m", kt=KD),
                wqk.ap().rearrange("(kt p) m -> p kt m", p=128),
            )
            w_v = wpool.tile([128, KD * HD], BF, tag="wv")
            nc.sync.dma_start(
                w_v[:].rearrange("p (kt m) -> p kt m", kt=KD),
                wv.ap().rearrange("(kt p) m -> p kt m", p=128),
            )
            w_p = wpool.tile([128, 2 * DIM], BF, tag="wp")
            nc.sync.dma_start(
                w_p[:].rearrange("p (kt m) -> p kt m", kt=2),
                wp.ap().rearrange("(kt p) m -> p kt m", p=128),
            )
            b_qk = small.tile([128, 4], F32, tag="bqk")
            nc.sync.dma_start(b_qk[:], bqk.ap().rearrange("(mt p) -> p mt", p=128))
            b_out = small.tile([128, 8], F32, tag="bout")
            nc.sync.dma_start(b_out[:], bout.ap().rearrange("(mt p) -> p mt", p=128))
            maskT_sb = small.tile([128, 128], BF, tag="maskT")
            nc.sync.dma_start(maskT_sb[:], maskT.ap())
            idb_sb = small.tile([128, 128], BF, tag="idb")
            nc.sync.dma_start(idb_sb[:], idb.ap())
            hsel_sb = small.tile([128, 2], BF, tag="hsel")
            nc.sync.dma_start(hsel_sb[:], hsel.ap())
            id_sb = small.tile([128, 128], F32, tag="ident")
            nc.sync.dma_start(id_sb[:], ident.ap())

            eps_ln = small.tile([128, 1], F32, tag="epsln")
            nc.gpsimd.memset(eps_ln[:], LN_EPS)
            eps_n = small.tile([128, 1], F32, tag="epsn")
            nc.gpsimd.memset(eps_n[:], 1e-24)

            with (
                tc.tile_pool(name="xpool", bufs=10) as xpool,
                tc.tile_pool(name="lnsq", bufs=2) as lnsqp,
                tc.tile_pool(name="yT", bufs=1) as yTp,
                tc.tile_pool(name="qkT", bufs=1) as qkTp,
                tc.tile_pool(name="vsb", bufs=1) as vsbp,
                tc.tile_pool(name="stat", bufs=1) as statp,
                tc.tile_pool(name="rows", bufs=6) as rowsp,
                tc.tile_pool(name="bc", bufs=4) as bcp,
            ):
                yT = yTp.tile([128, KD * N], BF, tag="yT")
                qkT = qkTp.tile([128, 4 * N], BF, tag="qkT")
                v_sb = vsbp.tile([128, NT * HPC * VW], BF, tag="v")
                nc.gpsimd.memset(v_sb[:], 1.0)
                pspool = tc.alloc_tile_pool(name="ps1", bufs=4, space="PSUM")

                # ---- LayerNorm: batched stats, then normalize + transpose ----
                sums = statp.tile([128, 16], F32, tag="sums")
                sumsq = statp.tile([128, 16], F32, tag="sumsq")
                mu = statp.tile([128, 16], F32, tag="mu")
                musq = statp.tile([128, 16], F32, tag="musq")
                var = statp.tile([128, 16], F32, tag="var")
                rstd = statp.tile([128, 16], F32, tag="rstd")
                for bb in range(2):
                    b8 = slice(bb * 8, bb * 8 + 8)
                    xts = []
                    for j8 in range(8):
                        m = bb * 8 + j8
                        xt = xpool.tile([128, DIM], F32, tag="x", name=f"x{m}")
                        xts.append(xt)
                        nc.sync.dma_start(
                            xt[:], xb.ap()[m * 128 : (m + 1) * 128, :]
                        )
                        nc.vector.reduce_sum(sums[:, m : m + 1], xt[:], axis=AX.X)
                        sq = lnsqp.tile([128, DIM], F32, tag="lnsq")
                        nc.scalar.activation(
                            sq[:], xt[:], AF.Square, accum_out=sumsq[:, m : m + 1]
                        )
                    nc.vector.tensor_scalar_mul(mu[:, b8], sums[:, b8], 1.0 / DIM)
                    nc.vector.tensor_tensor(
                        musq[:, b8], mu[:, b8], mu[:, b8], op=OP.mult
                    )
                    nc.vector.scalar_tensor_tensor(
                        var[:, b8], sumsq[:, b8], 1.0 / DIM, musq[:, b8],
                        op0=OP.mult, op1=OP.subtract,
                    )
                    nc.scalar.activation(
                        var[:, b8], var[:, b8], AF.Sqrt, bias=eps_ln[:]
                    )
                    nc.vector.reciprocal_approx_fast(rstd[:, b8], var[:, b8])
                    xbts = []
                    for j8 in range(8):
                        m = bb * 8 + j8
                        xb16 = xpool.tile([128, DIM], BF, tag="xb16", name=f"xb{m}")
                        xbts.append(xb16)
                        nc.vector.tensor_scalar(
                            xb16[:], xts[j8][:],
                            mu[:, m : m + 1], rstd[:, m : m + 1],
                            op0=OP.subtract, op1=OP.mult,
                        )
                    # transpose y -> yT (PE, bf16), 4 token tiles per copy
                    for kt in range(KD):
                        for mg in range(2):
                            pst = pspool.tile([128, 512], BF, tag="mm")
                            for j in range(4):
                                j8 = mg * 4 + j
                                m = bb * 8 + j8
                                nc.tensor.transpose(
                                    pst[:, j * 128 : (j + 1) * 128],
                                    xbts[j8][:, kt * 128 : (kt + 1) * 128],
                                    idb_sb[:],
                                )
                            nc.scalar.copy(
                                yT[
                                    :,
                                    kt * N + (bb * 2 + mg) * 512 :
                                    kt * N + (bb * 2 + mg + 1) * 512,
                                ],
                                pst[:],
                            )

                # ---- qk^T = W_qk^T @ y^T ----
                for c4 in range(4):
                    for dm in range(4):
                        ps = pspool.tile([128, 512], F32, tag="mm")
                        for kt in range(KD):
                            nc.tensor.matmul(
                                ps[:],
                                w_qk[:, kt * 512 + dm * 128 : kt * 512 + (dm + 1) * 128],
                                yT[:, kt * N + c4 * 512 : kt * N + (c4 + 1) * 512],
                                start=(kt == 0),
                                stop=(kt == KD - 1),
                            )
                        nc.scalar.activation(
                            qkT[:, dm * N + c4 * 512 : dm * N + (c4 + 1) * 512],
                            ps[:],
                            AF.Identity,
                            bias=b_qk[:, dm : dm + 1],
                        )

                # ---- v natural = y @ W_v, interleaved with ones columns ----
                for m in range(NT):
                    psv = pspool.tile([128, HD], F32, tag="mm")
                    for kt in range(KD):
                        nc.tensor.matmul(
                            psv[:],
                            yT[:, kt * N + m * 128 : kt * N + (m + 1) * 128],
                            w_v[:, kt * HD : (kt + 1) * HD],
                            start=(kt == 0),
                            stop=(kt == KD - 1),
                        )
                    vdst = v_sb[:].rearrange(
                        "p (mt h e) -> p mt h e", mt=NT, h=HPC
                    )[:, m, :, 0:DH]
                    nc.scalar.copy(
                        vdst, psv[:].rearrange("p (h e) -> p h e", h=HPC)
                    )

                # ---- L2 norms; normalize q AND k in place (bf16) ----
                for grp in (0, 2, 1, 3):
                    for c4 in range(4):
                        sqt = lnsqp.tile([128, 512], BF, tag="lnsqb")
                        nc.scalar.activation(
                            sqt[:],
                            qkT[:, grp * N + c4 * 512 : grp * N + (c4 + 1) * 512],
                            AF.Square,
                        )
                        psn = pspool.tile([2, 512], F32, tag="mm")
                        nc.tensor.matmul(
                            psn[:], hsel_sb[:], sqt[:], start=True, stop=True
                        )
                        srow = rowsp.tile([2, 512], F32, tag="srow")
                        nc.scalar.activation(
                            srow[:], psn[:], AF.Sqrt, bias=eps_n[0:2, :]
                        )
                        nc.vector.reciprocal_approx_fast(srow[:], srow[:])
                        for r in range(2):
                            prow = r * 64
                            src = srow[0:1, :]
                            if r == 1:
                                srow2 = rowsp.tile([2, 512], F32, tag="srow")
                                nc.sync.dma_start(srow2[0:1, :], srow[1:2, :])
                                src = srow2[0:1, :]
                            bc = bcp.tile([128, 512], F32, tag="bc")
                            nc.gpsimd.partition_broadcast(bc[:], src)
                            sl = qkT[
                                prow : prow + 64,
                                grp * N + c4 * 512 : grp * N + (c4 + 1) * 512,
                            ]
                            nc.vector.tensor_tensor(
                                sl, sl, bc[prow : prow + 64, :], op=OP.mult
                            )

                # ---- attention + c_proj + chunked reduce-scatter ----
                pspool.release()
                simpool = tc.alloc_tile_pool(name="pss", bufs=3, space="PSUM")
                opspool = tc.alloc_tile_pool(name="pso", bufs=2, space="PSUM")
                oT = qkTp.tile([128, 2 * N], BF, tag="oT")

                def attn_head(qc, h, exp_p, otmpp, rdp):
                    prow = (h % 2) * 64
                    qoff = (h // 2) * N
                    koff = (2 + h // 2) * N
                    pso = opspool.tile([65, 512], F32, tag="o")
                    nkt = 4 * qc + 4
                    for pg in range(nkt // 2):
                        pair = tuple(
                            (2 * pg + i, max(0, 2 * pg + i - 4 * qc) * 128)
                            for i in range(2)
                        )
                        pss = simpool.tile([128, 1024], F32, tag="sim")
                        for i, (kt, of) in enumerate(pair):
                            diag = kt >= 4 * qc
                            nc.tensor.matmul(
                                pss[:, i * 512 + of : (i + 1) * 512],
                                qkT[
                                    prow : prow + 64,
                                    koff + kt * 128 : koff + (kt + 1) * 128,
                                ],
                                qkT[
                                    prow : prow + 64,
                                    qoff + qc * 512 + of : qoff + (qc + 1) * 512,
                                ],
                                start=True,
                                stop=not diag,
                            )
                            if diag:
                                # sim += maskAdd on the diagonal block: exp of
                                # masked entries underflows to zero
                                db = i * 512 + of
                                nc.tensor.matmul(
                                    pss[:, db : db + 128],
                                    maskT_sb[:],
                                    idb_sb[:],
                                    start=False,
                                    stop=True,
                                )
                        of0 = pair[0][1]
                        ex = exp_p.tile([128, 1024], BF, tag="ex")
                        nc.scalar.activation(
                            ex[:, of0:1024], pss[:, of0:1024], AF.Exp, scale=SCALE
                        )
                        for i, (kt, of) in enumerate(pair):
                            nc.tensor.matmul(
                                pso[:, of:512],
                                v_sb[
                                    :,
                                    kt * HPC * VW + h * VW :
                                    kt * HPC * VW + h * VW + VW,
                                ],
                                ex[:, i * 512 + of : (i + 1) * 512],
                                start=(kt == 0),
                                stop=(kt == nkt - 1),
                            )
                    rdt = rdp.tile([65, 512], F32, tag="rd")
                    nc.scalar.copy(rdt[64:65, :], pso[64:65, :])
                    rd0 = rowsp.tile([2, 512], F32, tag="srow")
                    nc.sync.dma_start(rd0[0:1, :], rdt[64:65, :])
                    nc.vector.reciprocal_approx_fast(rd0[0:1, :], rd0[0:1, :])
                    bc = bcp.tile([128, 512], F32, tag="bc")
                    nc.gpsimd.partition_broadcast(bc[:], rd0[0:1, :])
                    if prow == 0:
                        nc.vector.tensor_tensor(
                            oT[0:64, qoff + qc * 512 : qoff + (qc + 1) * 512],
                            pso[0:64, :],
                            bc[0:64, :],
                            op=OP.mult,
                        )
                    else:
                        ot = otmpp.tile([64, 512], BF, tag="otmp")
                        nc.vector.tensor_tensor(
                            ot[:], pso[0:64, :], bc[0:64, :], op=OP.mult
                        )
                        nc.sync.dma_start(
                            oT[64:128, qoff + qc * 512 : qoff + (qc + 1) * 512],
                            ot[:],
                        )

                def proj_chunk(qc, projp):
                    for mm in range(4):
                        m = 4 * qc + mm
                        pj = projp.tile([128, DIM], BF, tag="proj")
                        for cc in range(2):
                            psp = simpool.tile([128, 512], F32, tag="sim")
                            for kt2 in range(2):
                                nc.tensor.matmul(
                                    psp[:],
                                    oT[:, kt2 * N + m * 128 : kt2 * N + (m + 1) * 128],
                                    w_p[:, kt2 * DIM + cc * 512 : kt2 * DIM + (cc + 1) * 512],
                                    start=(kt2 == 0),
                                    stop=(kt2 == 1),
                                )
                            nc.scalar.copy(pj[:, cc * 512 : (cc + 1) * 512], psp[:])
                        nc.sync.dma_start(
                            proj_c[qc].ap()[mm * 128 : (mm + 1) * 128, :], pj[:]
                        )
                    nc.gpsimd.collective_compute(
                        "ReduceScatter",
                        OP.add,
                        replica_groups=GROUPS,
                        ins=[proj_c[qc].ap().opt()],
                        outs=[rs_c[qc].ap().opt()],
                    )

                with tc.tile_pool(name="expool", bufs=4) as exp_p, tc.tile_pool(
                    name="otmp", bufs=2
                ) as otmpp, tc.tile_pool(name="rdp", bufs=2) as rdp, tc.tile_pool(
                    name="projp", bufs=3
                ) as projp:
                    for qc in range(4):
                        for h in range(HPC):
                            attn_head(qc, h, exp_p, otmpp, rdp)
                        proj_chunk(qc, projp)

                if DEBUG_DUMPS:
                    nc.sync.dma_start(d_qkT.ap(), qkT[:])
                    nc.sync.dma_start(d_v.ap(), v_sb[:])
                    nc.sync.dma_start(d_oT.ap(), oT[:])
                opspool.release()
                simpool.release()

            # ---- to_out on the token shard, one 128-token strip at a time ----
            with tc.tile_pool(name="tail", bufs=1) as tailp, tc.tile_pool(
                name="tout", bufs=2
            ) as toutp, tc.tile_pool(name="ps2", bufs=4, space="PSUM") as ps2:
                w_to = tailp.tile([128, KD * DIM], BF, tag="wto")
                nc.sync.dma_start(
                    w_to[:].rearrange("p (kt m) -> p kt m", kt=KD),
                    wto.ap().rearrange("(kt p) m -> p kt m", p=128),
                )
                for s in range(4):
                    rst = tailp.tile([128, DIM], BF, tag=f"rst{s}", name=f"rst{s}")
                    nc.sync.dma_start(rst[:], rs_c[s].ap())
                    rsT = tailp.tile(
                        [128, KD * 128], BF, tag=f"rsT{s}", name=f"rsT{s}"
                    )
                    nc.sync.dma_start_transpose(
                        rsT[:].rearrange("p (kt t) -> p kt t", kt=KD), rst[:]
                    )
                    for jm in range(KD):
                        pst = ps2.tile([128, 128], F32, tag="mm")
                        for kt in range(KD):
                            nc.tensor.matmul(
                                pst[:],
                                w_to[:, kt * DIM + jm * 128 : kt * DIM + (jm + 1) * 128],
                                rsT[:, kt * 128 : (kt + 1) * 128],
                                start=(kt == 0),
                                stop=(kt == KD - 1),
                            )
                        ot = toutp.tile([128, 128], F32, tag="ot")
                        nc.scalar.activation(
                            ot[:], pst[:], AF.Identity,
                            bias=b_out[:, jm : jm + 1],
                        )
                        nc.sync.dma_start(
                            out_ext.ap()[
                                jm * 128 : (jm + 1) * 128,
                                s * 128 : (s + 1) * 128,
                            ],
                            ot[:],
                        )

    nc.compile()
    return nc


_NC = None


def _get_nc():
    global _NC
    if _NC is None:
        _NC = _build_nc()
    return _NC


def _prep_inputs(x, g, b, c_attn_w, c_attn_b, c_proj_w, c_proj_b, to_out_w):
    """Host-side fold + shard.  Returns per-core input maps."""
    f32 = np.float32
    bf16 = ml_dtypes.bfloat16
    W = (g[:, None] * c_attn_w).astype(f32)          # [DIM, 3*DIM]
    cb = (c_attn_b + b @ c_attn_w).astype(f32)       # [3*DIM]
    cb_v = cb[2 * DIM :]
    b_out = ((c_proj_b + cb_v @ c_proj_w) @ to_out_w.T).astype(f32)
    w_to_T = np.ascontiguousarray(to_out_w.T).astype(bf16)
    # mask added onto sim in [k, q] layout: -100 where q < k (strictly lower)
    mask_add = (-100.0 * (1.0 - np.triu(np.ones((128, 128))))).astype(f32)
    maskT = np.ascontiguousarray(mask_add.T).astype(bf16)
    idb = np.eye(128).astype(bf16)
    hs2 = np.zeros((128, 2), dtype=bf16)
    hs2[:64, 0] = 1.0
    hs2[64:, 1] = 1.0
    ident = np.eye(128, dtype=f32)

    in_maps = []
    for c in range(N_CORES):
        bi, gi = c // 4, c % 4
        h0 = gi * HPC * DH                           # 256*gi
        w_qk = np.ascontiguousarray(
            np.concatenate(
                [W[:, h0 : h0 + HD], W[:, DIM + h0 : DIM + h0 + HD]], axis=1
            )
        ).astype(bf16)
        b_qk = np.ascontiguousarray(
            np.concatenate([cb[h0 : h0 + HD], cb[DIM + h0 : DIM + h0 + HD]]),
            dtype=f32,
        )
        w_v = np.ascontiguousarray(W[:, 2 * DIM + h0 : 2 * DIM + h0 + HD]).astype(bf16)
        w_p = np.ascontiguousarray(c_proj_w[h0 : h0 + HD, :]).astype(bf16)
        in_maps.append(
            {
                "xb": np.ascontiguousarray(x[bi], dtype=f32),
                "wqk": w_qk,
                "bqk": b_qk,
                "wv": w_v,
                "wp": w_p,
                "wto": w_to_T,
                "bout": b_out,
                "maskT": maskT,
                "idb": idb,
                "hsel": hs2,
                "ident": ident,
            }
        )
    return in_maps


def kernel(x, g, b, c_attn_w, c_attn_b, c_proj_w, c_proj_b, to_out_w, **kw):
    nc = _get_nc()
    in_maps = _prep_inputs(
        np.asarray(x), np.asarray(g), np.asarray(b), np.asarray(c_attn_w),
        np.asarray(c_attn_b), np.asarray(c_proj_w), np.asarray(c_proj_b),
        np.asarray(to_out_w),
    )
    res = run_bass_kernel_spmd(nc, in_maps, list(range(N_CORES)), **kw)
    out = np.empty((B, N, DIM), dtype=np.float32)
    for c in range(N_CORES):
        bi, gi = c // 4, c % 4
        o = res.results[c]["out"]                    # [DIM, NSH], strips of 128
        for s in range(4):
            out[bi, s * 512 + gi * 128 : s * 512 + (gi + 1) * 128, :] = (
                o[:, s * 128 : (s + 1) * 128].T
            )
    kernel.last_result = res
    return out


# revision 39
# speedup vs baseline: 1.2513x; 1.0080x over previous
"""Trainium2 Bass kernel for a cosine-sim causal attention block (8 NeuronCores).

Reference computation (single device):
  y   = LayerNorm(x) * g + b
  qkv = y @ c_attn_w + c_attn_b ; split q,k,v into 16 heads of 64
  qn, kn = l2norm(q), l2norm(k)
  attn = softmax(causal(8 * qn @ kn^T))
  o    = attn @ v  (merge heads)
  o    = o @ c_proj_w + c_proj_b
  out  = o @ to_out_w.T

Sharding: 2 batch groups x 4 cores; core owns one batch and 4 heads
(tensor-parallel c_attn columns / c_proj rows).  Chunked ReduceScatter of the
c_proj partials inside each group token-shards the output; each core then
runs to_out on its 512-token shard.  Host folds g/b into c_attn, the v-bias
and c_proj bias through to a single to_out bias, and reassembles shards.

Compute: bf16 matmul operands, fp32 accumulation/softmax statistics.
Attention runs in the transposed [k, q] layout so softmax + attn@v need no
on-chip transposes; softmax denominators come free from a ones-column
appended to v; q/k norms fold into broadcast multiplies; the causal mask is
added onto the sim PSUM by a small PE matmul (exp then underflows to zero).
"""

import sys

for _p in ("/opt/trn_rl_repo", "/root/.axon_site/_ro/trn_rl_repo"):
    if _p not in sys.path:
        sys.path.insert(0, _p)

import numpy as np
import ml_dtypes

from concourse import bass, bacc, tile, mybir
from concourse.bass_utils import run_bass_kernel_spmd

B, N, DIM, H, DH = 2, 2048, 1024, 16, 64
SCALE = 8.0
LN_EPS = 1e-5

N_CORES = 8
GROUPS = [[0, 1, 2, 3], [4, 5, 6, 7]]
HPC = 4            # heads per core
HD = HPC * DH      # 256 head dims per core
NSH = N // 4       # 512 tokens per core after reduce-scatter

F32 = mybir.dt.float32
BF = mybir.dt.bfloat16
AF = mybir.ActivationFunctionType
OP = mybir.AluOpType
AX = mybir.AxisListType

NT = N // 128      # 16 token tiles
KD = DIM // 128    # 8 dim tiles
VW = DH + 1        # per-head v block width incl ones column

DEBUG_DUMPS = False


def _build_nc():
    nc = bacc.Bacc(
        "TRN2", target_bir_lowering=False, debug=False, num_devices=N_CORES
    )

    xb = nc.dram_tensor("xb", [N, DIM], F32, kind="ExternalInput")
    wqk = nc.dram_tensor("wqk", [DIM, 512], BF, kind="ExternalInput")
    bqk = nc.dram_tensor("bqk", [512], F32, kind="ExternalInput")
    wv = nc.dram_tensor("wv", [DIM, HD], BF, kind="ExternalInput")
    wp = nc.dram_tensor("wp", [HD, DIM], BF, kind="ExternalInput")
    wto = nc.dram_tensor("wto", [DIM, DIM], BF, kind="ExternalInput")
    bout = nc.dram_tensor("bout", [DIM], F32, kind="ExternalInput")
    maskT = nc.dram_tensor("maskT", [128, 128], BF, kind="ExternalInput")
    idb = nc.dram_tensor("idb", [128, 128], BF, kind="ExternalInput")
    hsel = nc.dram_tensor("hsel", [128, 2], BF, kind="ExternalInput")
    ident = nc.dram_tensor("ident", [128, 128], F32, kind="ExternalInput")
    out_ext = nc.dram_tensor("out", [DIM, NSH], F32, kind="ExternalOutput")

    proj_c = [nc.dram_tensor(f"proj_b{c}", [NSH, DIM], BF) for c in range(4)]
    rs_c = [nc.dram_tensor(f"rs_b{c}", [128, DIM], BF) for c in range(4)]
    if DEBUG_DUMPS:
        d_qkT = nc.dram_tensor("d_qkT", [128, 4 * N], BF, kind="ExternalOutput")
        d_v = nc.dram_tensor("d_v", [128, NT * HPC * VW], BF, kind="ExternalOutput")
        d_oT = nc.dram_tensor("d_oT", [128, 2 * N], BF, kind="ExternalOutput")

    with tile.TileContext(nc) as tc:
        with (
            tc.tile_pool(name="wpool", bufs=1) as wpool,
            tc.tile_pool(name="small", bufs=1) as small,
        ):
            # ---- persistent weights ----
            w_qk = wpool.tile([128, KD * 512], BF, tag="wqk")
            nc.sync.dma_start(
                w_qk[:].rearrange("p (kt m) -> p kt m", kt=KD),
                wqk.ap().rearrange("(kt p) m -> p kt m", p=128),
            )
            w_v = wpool.tile([128, KD * HD], BF, tag="wv")
            nc.sync.dma_start(
                w_v[:].rearrange("p (kt m) -> p kt m", kt=KD),
                wv.ap().rearrange("(kt p) m -> p kt m", p=128),
            )
            w_p = wpool.tile([128, 2 * DIM], BF, tag="wp")
            nc.sync.dma_start(
                w_p[:].rearrange("p (kt m) -> p kt m", kt=2),
                wp.ap().rearrange("(kt p) m -> p kt m", p=128),
            )
            b_qk = small.tile([128, 4], F32, tag="bqk")
            nc.sync.dma_start(b_qk[:], bqk.ap().rearrange("(mt p) -> p mt", p=128))
            b_out = small.tile([128, 8], F32, tag="bout")
            nc.sync.dma_start(b_out[:], bout.ap().rearrange("(mt p) -> p mt", p=128))
            maskT_sb = small.tile([128, 128], BF, tag="maskT")
            nc.sync.dma_start(maskT_sb[:], maskT.ap())
            idb_sb = small.tile([128, 128], BF, tag="idb")
            nc.sync.dma_start(idb_sb[:], idb.ap())
            hsel_sb = small.tile([128, 2], BF, tag="hsel")
            nc.sync.dma_start(hsel_sb[:], hsel.ap())
            id_sb = small.tile([128, 128], F32, tag="ident")
            nc.sync.dma_start(id_sb[:], ident.ap())

            eps_ln = small.tile([128, 1], F32, tag="epsln")
            nc.gpsimd.memset(eps_ln[:], LN_EPS)
            eps_n = small.tile([128, 1], F32, tag="epsn")
            nc.gpsimd.memset(eps_n[:], 1e-24)

            with (
                tc.tile_pool(name="xpool", bufs=10) as xpool,
                tc.tile_pool(name="lnsq", bufs=2) as lnsqp,
                tc.tile_pool(name="yT", bufs=4) as yTp,
                tc.tile_pool(name="qkT", bufs=4) as qkTp,
                tc.tile_pool(name="vsb", bufs=4) as vsbp,
                tc.tile_pool(name="stat", bufs=1) as statp,
                tc.tile_pool(name="rows", bufs=6) as rowsp,
                tc.tile_pool(name="bc", bufs=4) as bcp,
            ):
                yT_c = [
                    yTp.tile([128, KD * 512], BF, tag="yT", name=f"yT{c}")
                    for c in range(4)
                ]
                qkT_c = [
                    qkTp.tile([128, 4 * 512], BF, tag="qkT", name=f"qkT{c}")
                    for c in range(4)
                ]
                v_c = [
                    vsbp.tile([128, 4 * HPC * VW], BF, tag="v", name=f"v{c}")
                    for c in range(4)
                ]
                for c in range(4):
                    nc.gpsimd.memset(v_c[c][:], 1.0)
                pspool = tc.alloc_tile_pool(name="ps1", bufs=4, space="PSUM")

                # ---- LayerNorm: batched stats, then normalize + transpose ----
                sums = statp.tile([128, 16], F32, tag="sums")
                sumsq = statp.tile([128, 16], F32, tag="sumsq")
                mu = statp.tile([128, 16], F32, tag="mu")
                musq = statp.tile([128, 16], F32, tag="musq")
                var = statp.tile([128, 16], F32, tag="var")
                rstd = statp.tile([128, 16], F32, tag="rstd")
                for bb in range(2):
                    b8 = slice(bb * 8, bb * 8 + 8)
                    xts = []
                    for j8 in range(8):
                        m = bb * 8 + j8
                        xt = xpool.tile([128, DIM], F32, tag="x", name=f"x{m}")
                        xts.append(xt)
                        nc.sync.dma_start(
                            xt[:], xb.ap()[m * 128 : (m + 1) * 128, :]
                        )
                        nc.vector.reduce_sum(sums[:, m : m + 1], xt[:], axis=AX.X)
                        sq = lnsqp.tile([128, DIM], F32, tag="lnsq")
                        nc.scalar.activation(
                            sq[:], xt[:], AF.Square, accum_out=sumsq[:, m : m + 1]
                        )
                    nc.vector.tensor_scalar_mul(mu[:, b8], sums[:, b8], 1.0 / DIM)
                    nc.vector.tensor_tensor(
                        musq[:, b8], mu[:, b8], mu[:, b8], op=OP.mult
                    )
                    nc.vector.scalar_tensor_tensor(
                        var[:, b8], sumsq[:, b8], 1.0 / DIM, musq[:, b8],
                        op0=OP.mult, op1=OP.subtract,
                    )
                    nc.scalar.activation(
                        var[:, b8], var[:, b8], AF.Sqrt, bias=eps_ln[:]
                    )
                    nc.vector.reciprocal_approx_fast(rstd[:, b8], var[:, b8])
                    xbts = []
                    for j8 in range(8):
                        m = bb * 8 + j8
                        xb16 = xpool.tile([128, DIM], BF, tag="xb16", name=f"xb{m}")
                        xbts.append(xb16)
                        nc.vector.tensor_scalar(
                            xb16[:], xts[j8][:],
                            mu[:, m : m + 1], rstd[:, m : m + 1],
                            op0=OP.subtract, op1=OP.mult,
                        )
                    # transpose y -> yT (PE, bf16), 4 token tiles per copy
                    for kt in range(KD):
                        for mg in range(2):
                            pst = pspool.tile([128, 512], BF, tag="mm")
                            for j in range(4):
                                j8 = mg * 4 + j
                                m = bb * 8 + j8
                                nc.tensor.transpose(
                                    pst[:, j * 128 : (j + 1) * 128],
                                    xbts[j8][:, kt * 128 : (kt + 1) * 128],
                                    idb_sb[:],
                                )
                            nc.scalar.copy(
                                yT_c[bb * 2 + mg][
                                    :, kt * 512 : (kt + 1) * 512
                                ],
                                pst[:],
                            )

                # ---- qk^T = W_qk^T @ y^T ----
                for c4 in range(4):
                    for dm in range(4):
                        ps = pspool.tile([128, 512], F32, tag="mm")
                        for kt in range(KD):
                            nc.tensor.matmul(
                                ps[:],
                                w_qk[:, kt * 512 + dm * 128 : kt * 512 + (dm + 1) * 128],
                                yT_c[c4][:, kt * 512 : (kt + 1) * 512],
                                start=(kt == 0),
                                stop=(kt == KD - 1),
                            )
                        nc.scalar.activation(
                            qkT_c[c4][:, dm * 512 : (dm + 1) * 512],
                            ps[:],
                            AF.Identity,
                            bias=b_qk[:, dm : dm + 1],
                        )

                # ---- v natural = y @ W_v, interleaved with ones columns ----
                for m in range(NT):
                    psv = pspool.tile([128, HD], F32, tag="mm")
                    for kt in range(KD):
                        nc.tensor.matmul(
                            psv[:],
                            yT_c[m // 4][:, kt * 512 + (m % 4) * 128 : kt * 512 + (m % 4 + 1) * 128],
                            w_v[:, kt * HD : (kt + 1) * HD],
                            start=(kt == 0),
                            stop=(kt == KD - 1),
                        )
                    vdst = v_c[m // 4][:].rearrange(
                        "p (mt h e) -> p mt h e", mt=4, h=HPC
                    )[:, m % 4, :, 0:DH]
                    nc.scalar.copy(
                        vdst, psv[:].rearrange("p (h e) -> p h e", h=HPC)
                    )

                # ---- L2 norms; normalize q AND k in place (bf16) ----
                for grp in (0, 2, 1, 3):
                    for c4 in range(4):
                        sqt = lnsqp.tile([128, 512], BF, tag="lnsqb")
                        nc.scalar.activation(
                            sqt[:],
                            qkT_c[c4][:, grp * 512 : (grp + 1) * 512],
                            AF.Square,
                        )
                        psn = pspool.tile([2, 512], F32, tag="mm")
                        nc.tensor.matmul(
                            psn[:], hsel_sb[:], sqt[:], start=True, stop=True
                        )
                        srow = rowsp.tile([2, 512], F32, tag="srow")
                        nc.scalar.activation(
                            srow[:], psn[:], AF.Sqrt, bias=eps_n[0:2, :]
                        )
                        nc.vector.reciprocal_approx_fast(srow[:], srow[:])
                        for r in range(2):
                            prow = r * 64
                            src = srow[0:1, :]
                            if r == 1:
                                srow2 = rowsp.tile([2, 512], F32, tag="srow")
                                nc.sync.dma_start(srow2[0:1, :], srow[1:2, :])
                                src = srow2[0:1, :]
                            bc = bcp.tile([128, 512], F32, tag="bc")
                            nc.gpsimd.partition_broadcast(bc[:], src)
                            sl = qkT_c[c4][
                                prow : prow + 64, grp * 512 : (grp + 1) * 512
                            ]
                            nc.vector.tensor_tensor(
                                sl, sl, bc[prow : prow + 64, :], op=OP.mult
                            )

                # ---- attention + c_proj + chunked reduce-scatter ----
                pspool.release()
                simpool = tc.alloc_tile_pool(name="pss", bufs=3, space="PSUM")
                opspool = tc.alloc_tile_pool(name="pso", bufs=2, space="PSUM")
                oT_c = [
                    qkTp.tile([128, 2 * 512], BF, tag="oT", name=f"oT{c}")
                    for c in range(4)
                ]

                def attn_head(qc, h, exp_p, otmpp, rdp):
                    prow = (h % 2) * 64
                    pso = opspool.tile([65, 512], F32, tag="o")
                    nkt = 4 * qc + 4
                    for pg in range(nkt // 2):
                        pair = tuple(
                            (2 * pg + i, max(0, 2 * pg + i - 4 * qc) * 128)
                            for i in range(2)
                        )
                        pss = simpool.tile([128, 1024], F32, tag="sim")
                        for i, (kt, of) in enumerate(pair):
                            diag = kt >= 4 * qc
                            nc.tensor.matmul(
                                pss[:, i * 512 + of : (i + 1) * 512],
                                qkT_c[kt // 4][
                                    prow : prow + 64,
                                    (2 + h // 2) * 512 + (kt % 4) * 128 :
                                    (2 + h // 2) * 512 + (kt % 4 + 1) * 128,
                                ],
                                qkT_c[qc][
                                    prow : prow + 64,
                                    (h // 2) * 512 + of : (h // 2 + 1) * 512,
                                ],
                                start=True,
                                stop=not diag,
                            )
                            if diag:
                                # sim += maskAdd on the diagonal block: exp of
                                # masked entries underflows to zero
                                db = i * 512 + of
                                nc.tensor.matmul(
                                    pss[:, db : db + 128],
                                    maskT_sb[:],
                                    idb_sb[:],
                                    start=False,
                                    stop=True,
                                )
                        of0 = pair[0][1]
                        ex = exp_p.tile([128, 1024], BF, tag="ex")
                        nc.scalar.activation(
                            ex[:, of0:1024], pss[:, of0:1024], AF.Exp, scale=SCALE
                        )
                        for i, (kt, of) in enumerate(pair):
                            nc.tensor.matmul(
                                pso[:, of:512],
                                v_c[kt // 4][
                                    :,
                                    ((kt % 4) * HPC + h) * VW :
                                    ((kt % 4) * HPC + h) * VW + VW,
                                ],
                                ex[:, i * 512 + of : (i + 1) * 512],
                                start=(kt == 0),
                                stop=(kt == nkt - 1),
                            )
                    rdt = rdp.tile([65, 512], F32, tag="rd")
                    nc.scalar.copy(rdt[64:65, :], pso[64:65, :])
                    rd0 = rowsp.tile([2, 512], F32, tag="srow")
                    nc.sync.dma_start(rd0[0:1, :], rdt[64:65, :])
                    nc.vector.reciprocal_approx_fast(rd0[0:1, :], rd0[0:1, :])
                    bc = bcp.tile([128, 512], F32, tag="bc")
                    nc.gpsimd.partition_broadcast(bc[:], rd0[0:1, :])
                    if prow == 0:
                        nc.vector.tensor_tensor(
                            oT_c[qc][0:64, (h // 2) * 512 : (h // 2 + 1) * 512],
                            pso[0:64, :],
                            bc[0:64, :],
                            op=OP.mult,
                        )
                    else:
                        ot = otmpp.tile([64, 512], BF, tag="otmp")
                        nc.vector.tensor_tensor(
                            ot[:], pso[0:64, :], bc[0:64, :], op=OP.mult
                        )
                        nc.sync.dma_start(
                            oT_c[qc][64:128, (h // 2) * 512 : (h // 2 + 1) * 512],
                            ot[:],
                        )

                def proj_chunk(qc, projp):
                    for mm in range(4):
                        pj = projp.tile([128, DIM], BF, tag="proj")
                        for cc in range(2):
                            psp = simpool.tile([128, 512], F32, tag="sim")
                            for kt2 in range(2):
                                nc.tensor.matmul(
                                    psp[:],
                                    oT_c[qc][:, kt2 * 512 + mm * 128 : kt2 * 512 + (mm + 1) * 128],
                                    w_p[:, kt2 * DIM + cc * 512 : kt2 * DIM + (cc + 1) * 512],
                                    start=(kt2 == 0),
                                    stop=(kt2 == 1),
                                )
                            nc.scalar.copy(pj[:, cc * 512 : (cc + 1) * 512], psp[:])
                        nc.sync.dma_start(
                            proj_c[qc].ap()[mm * 128 : (mm + 1) * 128, :], pj[:]
                        )
                    nc.gpsimd.collective_compute(
                        "ReduceScatter",
                        OP.add,
                        replica_groups=GROUPS,
                        ins=[proj_c[qc].ap().opt()],
                        outs=[rs_c[qc].ap().opt()],
                    )

                with tc.tile_pool(name="expool", bufs=4) as exp_p, tc.tile_pool(
                    name="otmp", bufs=2
                ) as otmpp, tc.tile_pool(name="rdp", bufs=2) as rdp, tc.tile_pool(
                    name="projp", bufs=3
                ) as projp:
                    for qc in range(4):
                        for h in range(HPC):
                            attn_head(qc, h, exp_p, otmpp, rdp)
                        proj_chunk(qc, projp)

                if DEBUG_DUMPS:
                    for c in range(4):
                        nc.sync.dma_start(
                            d_qkT.ap().rearrange("p (dm t) -> p dm t", dm=4)[
                                :, :, c * 512 : (c + 1) * 512
                            ],
                            qkT_c[c][:].rearrange("p (dm t) -> p dm t", dm=4),
                        )
                        nc.sync.dma_start(
                            d_v.ap().rearrange("p (c r) -> p c r", c=4)[:, c, :],
                            v_c[c][:],
                        )
                        nc.sync.dma_start(
                            d_oT.ap().rearrange("p (k t) -> p k t", k=2)[
                                :, :, c * 512 : (c + 1) * 512
                            ],
                            oT_c[c][:].rearrange("p (k t) -> p k t", k=2),
                        )
                opspool.release()
                simpool.release()

            # ---- to_out on the token shard, one 128-token strip at a time ----
            with tc.tile_pool(name="tail", bufs=1) as tailp, tc.tile_pool(
                name="tout", bufs=2
            ) as toutp, tc.tile_pool(name="ps2", bufs=4, space="PSUM") as ps2:
                w_to = tailp.tile([128, KD * DIM], BF, tag="wto")
                nc.sync.dma_start(
                    w_to[:].rearrange("p (kt m) -> p kt m", kt=KD),
                    wto.ap().rearrange("(kt p) m -> p kt m", p=128),
                )
                for s in range(4):
                    rst = tailp.tile([128, DIM], BF, tag=f"rst{s}", name=f"rst{s}")
                    nc.sync.dma_start(rst[:], rs_c[s].ap())
                    rsT = tailp.tile(
                        [128, KD * 128], BF, tag=f"rsT{s}", name=f"rsT{s}"
                    )
                    nc.sync.dma_start_transpose(
                        rsT[:].rearrange("p (kt t) -> p kt t", kt=KD), rst[:]
                    )
                    for jm in range(KD):
                        pst = ps2.tile([128, 128], F32, tag="mm")
                        for kt in range(KD):
                            nc.tensor.matmul(
                                pst[:],
                                w_to[:, kt * DIM + jm * 128 : kt * DIM + (jm + 1) * 128],
                                rsT[:, kt * 128 : (kt + 1) * 128],
                                start=(kt == 0),
                                stop=(kt == KD - 1),
                            )
                        ot = toutp.tile([128, 128], F32, tag="ot")
                        nc.scalar.activation(
                            ot[:], pst[:], AF.Identity,
                            bias=b_out[:, jm : jm + 1],
                        )
                        nc.sync.dma_start(
                            out_ext.ap()[
                                jm * 128 : (jm + 1) * 128,
                                s * 128 : (s + 1) * 128,
                            ],
                            ot[:],
                        )

    nc.compile()
    return nc


_NC = None


def _get_nc():
    global _NC
    if _NC is None:
        _NC = _build_nc()
    return _NC


def _prep_inputs(x, g, b, c_attn_w, c_attn_b, c_proj_w, c_proj_b, to_out_w):
    """Host-side fold + shard.  Returns per-core input maps."""
    f32 = np.float32
    bf16 = ml_dtypes.bfloat16
    W = (g[:, None] * c_attn_w).astype(f32)          # [DIM, 3*DIM]
    cb = (c_attn_b + b @ c_attn_w).astype(f32)       # [3*DIM]
    cb_v = cb[2 * DIM :]
    b_out = ((c_proj_b + cb_v @ c_proj_w) @ to_out_w.T).astype(f32)
    w_to_T = np.ascontiguousarray(to_out_w.T).astype(bf16)
    # mask added onto sim in [k, q] layout: -100 where q < k (strictly lower)
    mask_add = (-100.0 * (1.0 - np.triu(np.ones((128, 128))))).astype(f32)
    maskT = np.ascontiguousarray(mask_add.T).astype(bf16)
    idb = np.eye(128).astype(bf16)
    hs2 = np.zeros((128, 2), dtype=bf16)
    hs2[:64, 0] = 1.0
    hs2[64:, 1] = 1.0
    ident = np.eye(128, dtype=f32)

    in_maps = []
    for c in range(N_CORES):
        bi, gi = c // 4, c % 4
        h0 = gi * HPC * DH                           # 256*gi
        w_qk = np.ascontiguousarray(
            np.concatenate(
                [W[:, h0 : h0 + HD], W[:, DIM + h0 : DIM + h0 + HD]], axis=1
            )
        ).astype(bf16)
        b_qk = np.ascontiguousarray(
            np.concatenate([cb[h0 : h0 + HD], cb[DIM + h0 : DIM + h0 + HD]]),
            dtype=f32,
        )
        w_v = np.ascontiguousarray(W[:, 2 * DIM + h0 : 2 * DIM + h0 + HD]).astype(bf16)
        w_p = np.ascontiguousarray(c_proj_w[h0 : h0 + HD, :]).astype(bf16)
        in_maps.append(
            {
                "xb": np.ascontiguousarray(x[bi], dtype=f32),
                "wqk": w_qk,
                "bqk": b_qk,
                "wv": w_v,
                "wp": w_p,
                "wto": w_to_T,
                "bout": b_out,
                "maskT": maskT,
                "idb": idb,
                "hsel": hs2,
                "ident": ident,
            }
        )
    return in_maps


def kernel(x, g, b, c_attn_w, c_attn_b, c_proj_w, c_proj_b, to_out_w, **kw):
    nc = _get_nc()
    in_maps = _prep_inputs(
        np.asarray(x), np.asarray(g), np.asarray(b), np.asarray(c_attn_w),
        np.asarray(c_attn_b), np.asarray(c_proj_w), np.asarray(c_proj_b),
        np.asarray(to_out_w),
    )
    res = run_bass_kernel_spmd(nc, in_maps, list(range(N_CORES)), **kw)
    out = np.empty((B, N, DIM), dtype=np.float32)
    for c in range(N_CORES):
        bi, gi = c // 4, c % 4
        o = res.results[c]["out"]                    # [DIM, NSH], strips of 128
        for s in range(4):
            out[bi, s * 512 + gi * 128 : s * 512 + (gi + 1) * 128, :] = (
                o[:, s * 128 : (s + 1) * 128].T
            )
    kernel.last_result = res
    return out
